# revision 26
# baseline (speedup 1.0000x reference)
"""Trainium2 Bass kernel for nn_Basic_MPNN — v6 (block-diagonal mask, balanced drains).

One fp8 DoubleRow matmul pair-span per 2 senders computes masked messages:
k-tile 0 contracts We with the edge chunk; k-tile 1 contracts a per-chunk
shared lhsT block (m2q/m2r/-224 rows for 16 senders at partitions 3q..3q+2)
against a block-diagonal adjacency rhs (gate/gate/1-gate at the same rows).
All m2/mask setup is host-packed: no on-device memsets of the weight side,
no on-device m2 matmuls.  Drains: Act copies 48 of 64 PSUM groups to f16
leaves (DVE folds them), DVE direct-folds the other 16 (max(acc, psum)),
sized from the cost model's LP optimum (Pool cannot read PSUM on TRN2 and
tensor ops allow only one PSUM operand, so these are the only two exits).
"""

import os
import sys

for _p in (
    "/root/.axon_site",
    "/root/.axon_site/_ro/trn_rl_repo",
    "/root/.axon_site/_ro/pypackages",
    "/opt/trn_rl_repo",
    "/opt/pypackages",
):
    if os.path.isdir(_p) and _p not in sys.path:
        sys.path.append(_p)

import numpy as np  # noqa: E402

import concourse.bass as bass  # noqa: E402
import concourse.tile as tile  # noqa: E402
from concourse import bacc, mybir  # noqa: E402
from concourse.ap import AP as BassAP  # noqa: E402
from concourse.bass_utils import run_bass_kernel_spmd  # noqa: E402

F32 = mybir.dt.float32
F16 = mybir.dt.float16
F8 = mybir.dt.float8e4

B, N, D, MID, OUT = 4, 512, 128, 128, 128
NCORES = 8
IH = N // 2            # receivers per core
JD = 16                # senders per edge chunk
NCHUNK = N // JD       # 32
CW = JD * IH           # 4096 edge cols per chunk
WEMW = 128 + NCHUNK * 128   # We + one 128-col m2 block per chunk
FINW = 896             # noderT(256) wo1(128) wo2(128) cT(256) row0:bso(128)
MASK_NEG = -224.0
EBUFS = 3
GSEND = 4              # senders per PSUM drain group
NGRP = N // GSEND      # 128
GW = GSEND * IH        # 1024 psum cols per group (4 senders x 256 receivers)
# drain classes (Pool cannot read PSUM; DVE ops allow only ONE PSUM operand):
#   AP: Act copy [2048]->f16 leaf, Pool folds into its acc chain
#   AD: Act copy, DVE folds
#   D:  DVE folds PSUM directly into f16 acc (max(acc, ps), fold inlined)
# LP balance from the cost model: 25 AP / 14 AD / 25 D.
_QUOTA = (("AD", 93.0), ("D", 35.0))


def _class_seq():
    acc = {k: 0.0 for k, _ in _QUOTA}
    seq = []
    for _ in range(NGRP):
        for k, q in _QUOTA:
            acc[k] += q / NGRP
        pick = max(acc, key=lambda k: acc[k])
        acc[pick] -= 1.0
        seq.append(pick)
    return seq


CLASS_SEQ = _class_seq()
LAST_AD = max(i for i, k in enumerate(CLASS_SEQ) if k == "AD")


def _build_program():
    nc = bacc.Bacc(
        "TRN2", target_bir_lowering=False, debug=False, num_devices=NCORES
    )

    edge = nc.dram_tensor("edge", [D, N, IH], F8, kind="ExternalInput").ap()
    adjf_d = nc.dram_tensor(
        "adjf", [NCHUNK * 3 * JD, CW], F8, kind="ExternalInput"
    ).ap()
    wem_d = nc.dram_tensor("wem", [128, WEMW], F8, kind="ExternalInput").ap()
    fin_d = nc.dram_tensor("finpack", [128, FINW], F16, kind="ExternalInput").ap()
    zeros_d = nc.dram_tensor("zeros", [128, CW], F8, kind="ExternalInput").ap()
    out_d = nc.dram_tensor("out", [IH, OUT], F32, kind="ExternalOutput").ap()

    with (
        tile.TileContext(nc) as tc,
        tc.tile_pool(name="persist", bufs=1) as pp,
        tc.tile_pool(name="edge", bufs=EBUFS) as ep,
        tc.tile_pool(name="leafA", bufs=8) as lAp,
        tc.tile_pool(name="accAD", bufs=2) as aDp,
        tc.tile_pool(name="accD", bufs=2) as aDDp,
        tc.tile_pool(name="fin", bufs=8) as fp,
        tc.tile_pool(name="ps2", bufs=4, space="PSUM") as psp,
    ):
        wem_sb = pp.tile([128, WEMW], F8)
        fin_sb = pp.tile([128, FINW], F16)
        ones16 = pp.tile([1, 128], F16)
        nc.gpsimd.memset(ones16[:], 1.0)

        noderT16 = fin_sb[:, 0:256]
        wo1_16 = fin_sb[:, 256:384]
        wo2_16 = fin_sb[:, 384:512]
        cT16 = fin_sb[:, 512:768]
        bso16 = fin_sb[0:1, 768:896]

        wb = wem_sb[:]
        pitch = wb.ap[0][0]

        # two independent fold chains on DVE
        accAD = [None]   # Act leaves folded by DVE
        accD = [None]    # PSUM folded directly by DVE
        preAD = [None]

        for c in range(NCHUNK):
            et = ep.tile([128, 2 * CW], F8, tag="e")
            # adjacency region init: off-diagonal + rows>=48 must be 0 once
            # per buffer (diagonal blocks are rewritten by every chunk's DMA)
            if c == 0:
                nc.scalar.memzero(et[:, CW:CW + 2048])
                nc.gpsimd.memset(et[:, CW + 2048:2 * CW], 0.0)
            elif c == 1:
                nc.gpsimd.memset(et[:, CW:CW + 2048], 0.0)
                nc.scalar.memzero(et[:, CW + 2048:2 * CW])
            elif c == 2:
                nc.scalar.dma_start(et[:, CW:2 * CW], zeros_d[:, :])
            nc.sync.dma_start(
                et[:, 0:CW],
                edge[:, c * JD:(c + 1) * JD, :].rearrange("d j i -> d (j i)"),
            )
            ett = et[:]
            nc.sync.dma_start(
                et[0:3 * JD, CW:2 * CW],
                adjf_d[c * 3 * JD:(c + 1) * 3 * JD, :],
            )
            if c == 0:
                # after chunk-0 stream DMAs so the first edge tile lands early
                nc.sync.dma_start(wem_sb[:], wem_d[:, :])
            elif c == 16:
                nc.scalar.dma_start(fin_sb[:], fin_d[:, :])
            et2 = ett.rearrange("d (t x) -> d t x", t=2)
            lhsT = BassAP(
                wb.tensor, wb.offset,
                [[pitch, 128], [128 * (c + 1), 2], [1, 128]],
            )
            for h in range(4):
                ps = psp.tile([128, GW], F32, tag="ps")
                for m in range(2):
                    s = h * GW + m * 512
                    nc.tensor.matmul(
                        ps[:, m * 512:(m + 1) * 512],
                        lhsT=lhsT,
                        rhs=et2[:, :, s:s + 512],
                        perf_mode=mybir.MatmulPerfMode.DoubleRow,
                        start=True, stop=True,
                    )
                g = 4 * c + h
                cls = CLASS_SEQ[g]
                if cls == "D":
                    nd = aDDp.tile([128, GW], F16, tag="accD")
                    if accD[0] is None:
                        nc.vector.tensor_copy(nd[:], ps[:])
                    else:
                        nc.vector.tensor_max(nd[:], accD[0][:], ps[:])
                    accD[0] = nd
                else:
                    if accAD[0] is None:
                        leaf = aDp.tile([128, GW], F16, tag="acc")
                        nc.scalar.copy(leaf[:], ps[:])
                        accAD[0] = leaf
                    else:
                        leaf = lAp.tile([128, GW], F16, tag="lA")
                        nc.scalar.copy(leaf[:], ps[:])
                        na = aDp.tile([128, GW], F16, tag="acc")
                        nc.vector.tensor_max(na[:], accAD[0][:], leaf[:])
                        accAD[0] = na
                if g == LAST_AD and g < NGRP - 1:
                    # pre-halve the Act acc while the trailing groups drain
                    tAD = fp.tile([128, 512], F16, tag="fAD")
                    nc.vector.tensor_max(
                        tAD[:], accAD[0][:, 0:512], accAD[0][:, 512:1024]
                    )
                    preAD[0] = tAD

        # ---------------- merge + finalize ----------------
        if preAD[0] is None:
            preAD[0] = fp.tile([128, 512], F16, tag="fAD", name="preAD0")
            nc.vector.tensor_max(
                preAD[0][:], accAD[0][:, 0:512], accAD[0][:, 512:1024]
            )
        m1 = fp.tile([128, 512], F16, tag="f")
        nc.vector.tensor_max(m1[:], preAD[0][:], accD[0][:, 0:512])
        m2 = fp.tile([128, 512], F16, tag="f")
        nc.vector.tensor_max(m2[:], m1[:], accD[0][:, 512:1024])
        w = fp.tile([128, 256], F16, tag="f")
        nc.vector.tensor_max(w[:], m2[:, 0:256], m2[:, 256:512])
        msgs = fp.tile([128, 256], F16, tag="f")
        nc.vector.tensor_add(msgs[:], w[:], cT16)

        for ib in range(2):
            psf = psp.tile([128, GW], F32, tag="ps")
            ps_h = psf[:, 0:OUT]
            nc.tensor.matmul(
                ps_h, lhsT=msgs[:, ib * 128:(ib + 1) * 128],
                rhs=wo2_16, start=True, stop=False,
            )
            nc.tensor.matmul(
                ps_h, lhsT=noderT16[:, ib * 128:(ib + 1) * 128],
                rhs=wo1_16, start=False, stop=False,
            )
            nc.tensor.matmul(
                ps_h, lhsT=ones16[:, 0:128], rhs=bso16,
                start=False, stop=True,
            )
            o_sb = fp.tile([128, OUT], F32, tag="o")
            nc.scalar.activation(
                o_sb[:], ps_h, mybir.ActivationFunctionType.Relu
            )
            nc.sync.dma_start(out_d[ib * 128:(ib + 1) * 128, :], o_sb[:])

    nc.finalize()
    return nc


_CACHED = {}


def _get_program():
    if "nc" not in _CACHED:
        _CACHED["nc"] = _build_program()
    return _CACHED["nc"]


def kernel(**inputs) -> np.ndarray:
    import ml_dtypes
    F8NP = ml_dtypes.float8_e4m3

    nc = _get_program()

    def f32(x):
        return np.ascontiguousarray(np.asarray(x, dtype=np.float32))

    node_fts = f32(inputs["node_fts"])
    graph_fts = f32(inputs["graph_fts"])
    adj01 = np.asarray(inputs["adj_mat"]).astype(np.float32)
    edge8 = np.asarray(inputs["edge_fts"], dtype=F8NP)
    edgeT = edge8.transpose(0, 3, 1, 2)  # [B, D, j, i] view

    W1, b1 = f32(inputs["W1"]), f32(inputs["b1"])
    W2, b2 = f32(inputs["W2"]), f32(inputs["b2"])
    We8 = np.asarray(inputs["We"], dtype=F8NP)
    be = f32(inputs["be"])
    Wg, bg = f32(inputs["Wg"]), f32(inputs["bg"])
    Wo1, bo1 = f32(inputs["Wo1"]), f32(inputs["bo1"])
    Wo2, bo2 = f32(inputs["Wo2"]), f32(inputs["bo2"])

    zeros = np.zeros((128, CW), dtype=F8NP)
    in_maps = []
    for c in range(NCORES):
        bb, ihh = c // 2, c % 2
        sl = slice(ihh * IH, (ihh + 1) * IH)
        m = {}
        m["edge"] = np.ascontiguousarray(edgeT[bb, :, :, sl])
        m["zeros"] = zeros

        # m2 = sender-side message part; split into fp8 value + residual
        m2 = node_fts[bb] @ W2 + b2                    # [N, MID] f32
        m2q = m2.astype(F8NP)
        m2r = (m2 - m2q.astype(np.float32)).astype(F8NP)
        blocks = np.zeros((NCHUNK, 128, 128), dtype=F8NP)
        qi = np.arange(JD)
        blocks[:, 3 * qi + 0, :] = m2q.reshape(NCHUNK, JD, MID)
        blocks[:, 3 * qi + 1, :] = m2r.reshape(NCHUNK, JD, MID)
        blocks[:, 3 * qi + 2, :] = np.float32(MASK_NEG).astype(F8NP)
        wem = np.zeros((128, WEMW), dtype=F8NP)
        wem[:, 0:128] = We8
        wem[:, 128:] = np.ascontiguousarray(
            blocks.transpose(1, 0, 2)
        ).reshape(128, NCHUNK * 128)
        m["wem"] = wem

        # block-diagonal adjacency rows: gate / gate / 1-gate for sender q at
        # partitions 3q..3q+2, columns q*IH..(q+1)*IH; zeros elsewhere
        g = adj01[bb][:, sl]                           # [N, IH]
        gr = g.reshape(NCHUNK, JD, IH).astype(F8NP)
        gn = (1.0 - g).reshape(NCHUNK, JD, IH).astype(F8NP)
        adjf = np.zeros((NCHUNK, 3 * JD, CW), dtype=F8NP)
        for q in range(JD):
            adjf[:, 3 * q + 0, q * IH:(q + 1) * IH] = gr[:, q]
            adjf[:, 3 * q + 1, q * IH:(q + 1) * IH] = gr[:, q]
            adjf[:, 3 * q + 2, q * IH:(q + 1) * IH] = gn[:, q]
        m["adjf"] = adjf.reshape(NCHUNK * 3 * JD, CW)

        # receiver-side constant cT = (node@W1 + graph@Wg + b1+be+bg)^T
        cT = (node_fts[bb, sl] @ W1 + graph_fts[bb] @ Wg + b1 + be + bg).T
        fin = np.zeros((128, FINW), dtype=np.float16)
        fin[:, 0:256] = node_fts[bb, sl].T
        fin[:, 256:384] = Wo1
        fin[:, 384:512] = Wo2
        fin[:, 512:768] = cT
        fin[0, 768:896] = bo1 + bo2
        m["finpack"] = fin
        in_maps.append(m)

    res = run_bass_kernel_spmd(nc, in_maps, list(range(NCORES)))

    out = np.empty((B, N, OUT), dtype=np.float32)
    for c in range(NCORES):
        bb, ihh = c // 2, c % 2
        out[bb, ihh * IH:(ihh + 1) * IH, :] = res.results[c]["out"]
    return out


# revision 27
# speedup vs baseline: 1.0047x; 1.0047x over previous
"""Trainium2 Bass kernel for nn_Basic_MPNN — v6 (block-diagonal mask, balanced drains).

One fp8 DoubleRow matmul pair-span per 2 senders computes masked messages:
k-tile 0 contracts We with the edge chunk; k-tile 1 contracts a per-chunk
shared lhsT block (m2q/m2r/-224 rows for 16 senders at partitions 3q..3q+2)
against a block-diagonal adjacency rhs (gate/gate/1-gate at the same rows).
All m2/mask setup is host-packed: no on-device memsets of the weight side,
no on-device m2 matmuls.  Drains: Act copies 48 of 64 PSUM groups to f16
leaves (DVE folds them), DVE direct-folds the other 16 (max(acc, psum)),
sized from the cost model's LP optimum (Pool cannot read PSUM on TRN2 and
tensor ops allow only one PSUM operand, so these are the only two exits).
"""

import os
import sys

for _p in (
    "/root/.axon_site",
    "/root/.axon_site/_ro/trn_rl_repo",
    "/root/.axon_site/_ro/pypackages",
    "/opt/trn_rl_repo",
    "/opt/pypackages",
):
    if os.path.isdir(_p) and _p not in sys.path:
        sys.path.append(_p)

import numpy as np  # noqa: E402

import concourse.bass as bass  # noqa: E402
import concourse.tile as tile  # noqa: E402
from concourse import bacc, mybir  # noqa: E402
from concourse.ap import AP as BassAP  # noqa: E402
from concourse.bass_utils import run_bass_kernel_spmd  # noqa: E402

F32 = mybir.dt.float32
F16 = mybir.dt.float16
F8 = mybir.dt.float8e4

B, N, D, MID, OUT = 4, 512, 128, 128, 128
NCORES = 8
IH = N // 2            # receivers per core
JD = 16                # senders per edge chunk
NCHUNK = N // JD       # 32
CW = JD * IH           # 4096 edge cols per chunk
WEMW = 128 + NCHUNK * 128   # We + one 128-col m2 block per chunk
FINW = 896             # noderT(256) wo1(128) wo2(128) cT(256) row0:bso(128)
MASK_NEG = -224.0
EBUFS = 3
GSEND = 8              # senders per PSUM drain group
NGRP = N // GSEND      # 64
GW = GSEND * IH        # 2048 psum cols per group (8 senders x 256 receivers)
# drain classes (Pool cannot read PSUM; DVE ops allow only ONE PSUM operand):
#   AP: Act copy [2048]->f16 leaf, Pool folds into its acc chain
#   AD: Act copy, DVE folds
#   D:  DVE folds PSUM directly into f16 acc (max(acc, ps), fold inlined)
# LP balance from the cost model: 25 AP / 14 AD / 25 D.
_QUOTA = (("AD", 47.0), ("D", 17.0))


def _class_seq():
    acc = {k: 0.0 for k, _ in _QUOTA}
    seq = []
    for _ in range(NGRP):
        for k, q in _QUOTA:
            acc[k] += q / NGRP
        pick = max(acc, key=lambda k: acc[k])
        acc[pick] -= 1.0
        seq.append(pick)
    return seq


CLASS_SEQ = _class_seq()
LAST_AD = max(i for i, k in enumerate(CLASS_SEQ) if k == "AD")


def _build_program():
    nc = bacc.Bacc(
        "TRN2", target_bir_lowering=False, debug=False, num_devices=NCORES
    )

    edge = nc.dram_tensor("edge", [D, N, IH], F8, kind="ExternalInput").ap()
    adjf_d = nc.dram_tensor(
        "adjf", [NCHUNK * 3 * JD, CW], F8, kind="ExternalInput"
    ).ap()
    wem_d = nc.dram_tensor("wem", [128, WEMW], F8, kind="ExternalInput").ap()
    fin_d = nc.dram_tensor("finpack", [128, FINW], F16, kind="ExternalInput").ap()
    zeros_d = nc.dram_tensor("zeros", [128, CW], F8, kind="ExternalInput").ap()
    out_d = nc.dram_tensor("out", [IH, OUT], F32, kind="ExternalOutput").ap()

    with (
        tile.TileContext(nc) as tc,
        tc.tile_pool(name="persist", bufs=1) as pp,
        tc.tile_pool(name="edge", bufs=EBUFS) as ep,
        tc.tile_pool(name="leafA", bufs=8) as lAp,
        tc.tile_pool(name="accAD", bufs=2) as aDp,
        tc.tile_pool(name="accD", bufs=2) as aDDp,
        tc.tile_pool(name="fin", bufs=8) as fp,
        tc.tile_pool(name="ps2", bufs=2, space="PSUM") as psp,
    ):
        wem_sb = pp.tile([128, WEMW], F8)
        fin_sb = pp.tile([128, FINW], F16)
        ones16 = pp.tile([1, 128], F16)
        nc.gpsimd.memset(ones16[:], 1.0)

        noderT16 = fin_sb[:, 0:256]
        wo1_16 = fin_sb[:, 256:384]
        wo2_16 = fin_sb[:, 384:512]
        cT16 = fin_sb[:, 512:768]
        bso16 = fin_sb[0:1, 768:896]

        wb = wem_sb[:]
        pitch = wb.ap[0][0]

        # two independent fold chains on DVE
        accAD = [None]   # Act leaves folded by DVE
        accD = [None]    # PSUM folded directly by DVE
        preAD = [None]

        for c in range(NCHUNK):
            et = ep.tile([128, 2 * CW], F8, tag="e")
            # adjacency region init: off-diagonal + rows>=48 must be 0 once
            # per buffer (diagonal blocks are rewritten by every chunk's DMA)
            if c == 0:
                nc.vector.memset(et[:, CW:CW + 2048], 0.0)
                nc.gpsimd.memset(et[:, CW + 2048:2 * CW], 0.0)
            elif c == 1:
                nc.gpsimd.memset(et[:, CW:CW + 2048], 0.0)
                nc.vector.memset(et[:, CW + 2048:2 * CW], 0.0)
            elif c == 2:
                nc.scalar.dma_start(et[:, CW:2 * CW], zeros_d[:, :])
            nc.sync.dma_start(
                et[:, 0:CW],
                edge[:, c * JD:(c + 1) * JD, :].rearrange("d j i -> d (j i)"),
            )
            ett = et[:]
            nc.sync.dma_start(
                et[0:3 * JD, CW:2 * CW],
                adjf_d[c * 3 * JD:(c + 1) * 3 * JD, :],
            )
            if c == 0:
                # after chunk-0 stream DMAs so the first edge tile lands early
                nc.sync.dma_start(wem_sb[:], wem_d[:, :])
            elif c == 16:
                nc.scalar.dma_start(fin_sb[:], fin_d[:, :])
            et2 = ett.rearrange("d (t x) -> d t x", t=2)
            lhsT = BassAP(
                wb.tensor, wb.offset,
                [[pitch, 128], [128 * (c + 1), 2], [1, 128]],
            )
            for h in range(2):
                ps = psp.tile([128, GW], F32, tag="ps")
                for m in range(4):
                    s = h * GW + m * 512
                    nc.tensor.matmul(
                        ps[:, m * 512:(m + 1) * 512],
                        lhsT=lhsT,
                        rhs=et2[:, :, s:s + 512],
                        perf_mode=mybir.MatmulPerfMode.DoubleRow,
                        start=True, stop=True,
                    )
                g = 2 * c + h
                cls = CLASS_SEQ[g]
                if cls == "D":
                    nd = aDDp.tile([128, GW], F16, tag="accD")
                    if accD[0] is None:
                        nc.vector.tensor_copy(nd[:], ps[:])
                    else:
                        nc.vector.tensor_max(nd[:], accD[0][:], ps[:])
                    accD[0] = nd
                else:
                    if accAD[0] is None:
                        leaf = aDp.tile([128, GW], F16, tag="acc")
                        nc.scalar.copy(leaf[:], ps[:])
                        accAD[0] = leaf
                    else:
                        leaf = lAp.tile([128, GW], F16, tag="lA")
                        nc.scalar.copy(leaf[:], ps[:])
                        na = aDp.tile([128, GW], F16, tag="acc")
                        nc.vector.tensor_max(na[:], accAD[0][:], leaf[:])
                        accAD[0] = na
                if g == LAST_AD and g < NGRP - 1:
                    # pre-halve the Act acc while the trailing groups drain
                    tAD = fp.tile([128, 1024], F16, tag="fAD")
                    nc.vector.tensor_max(
                        tAD[:], accAD[0][:, 0:1024], accAD[0][:, 1024:2048]
                    )
                    preAD[0] = tAD

        # ---------------- merge + finalize ----------------
        if preAD[0] is None:
            preAD[0] = fp.tile([128, 1024], F16, tag="fAD", name="preAD0")
            nc.vector.tensor_max(
                preAD[0][:], accAD[0][:, 0:1024], accAD[0][:, 1024:2048]
            )
        m1 = fp.tile([128, 1024], F16, tag="f")
        nc.vector.tensor_max(m1[:], preAD[0][:], accD[0][:, 0:1024])
        m2 = fp.tile([128, 1024], F16, tag="f")
        nc.vector.tensor_max(m2[:], m1[:], accD[0][:, 1024:2048])
        t3 = fp.tile([128, 512], F16, tag="f")
        nc.vector.tensor_max(t3[:], m2[:, 0:512], m2[:, 512:1024])
        w = fp.tile([128, 256], F16, tag="f")
        nc.vector.tensor_max(w[:], t3[:, 0:256], t3[:, 256:512])
        msgs = fp.tile([128, 256], F16, tag="f")
        nc.vector.tensor_add(msgs[:], w[:], cT16)

        for ib in range(2):
            psf = psp.tile([128, GW], F32, tag="ps")
            ps_h = psf[:, 0:OUT]
            nc.tensor.matmul(
                ps_h, lhsT=msgs[:, ib * 128:(ib + 1) * 128],
                rhs=wo2_16, start=True, stop=False,
            )
            nc.tensor.matmul(
                ps_h, lhsT=noderT16[:, ib * 128:(ib + 1) * 128],
                rhs=wo1_16, start=False, stop=False,
            )
            nc.tensor.matmul(
                ps_h, lhsT=ones16[:, 0:128], rhs=bso16,
                start=False, stop=True,
            )
            o_sb = fp.tile([128, OUT], F32, tag="o")
            nc.scalar.activation(
                o_sb[:], ps_h, mybir.ActivationFunctionType.Relu
            )
            nc.sync.dma_start(out_d[ib * 128:(ib + 1) * 128, :], o_sb[:])

    nc.finalize()
    return nc


_CACHED = {}


def _get_program():
    if "nc" not in _CACHED:
        _CACHED["nc"] = _build_program()
    return _CACHED["nc"]


def kernel(**inputs) -> np.ndarray:
    import ml_dtypes
    F8NP = ml_dtypes.float8_e4m3

    nc = _get_program()

    def f32(x):
        return np.ascontiguousarray(np.asarray(x, dtype=np.float32))

    node_fts = f32(inputs["node_fts"])
    graph_fts = f32(inputs["graph_fts"])
    adj01 = np.asarray(inputs["adj_mat"]).astype(np.float32)
    edge8 = np.asarray(inputs["edge_fts"], dtype=F8NP)
    edgeT = edge8.transpose(0, 3, 1, 2)  # [B, D, j, i] view

    W1, b1 = f32(inputs["W1"]), f32(inputs["b1"])
    W2, b2 = f32(inputs["W2"]), f32(inputs["b2"])
    We8 = np.asarray(inputs["We"], dtype=F8NP)
    be = f32(inputs["be"])
    Wg, bg = f32(inputs["Wg"]), f32(inputs["bg"])
    Wo1, bo1 = f32(inputs["Wo1"]), f32(inputs["bo1"])
    Wo2, bo2 = f32(inputs["Wo2"]), f32(inputs["bo2"])

    zeros = np.zeros((128, CW), dtype=F8NP)
    in_maps = []
    for c in range(NCORES):
        bb, ihh = c // 2, c % 2
        sl = slice(ihh * IH, (ihh + 1) * IH)
        m = {}
        m["edge"] = np.ascontiguousarray(edgeT[bb, :, :, sl])
        m["zeros"] = zeros

        # m2 = sender-side message part; split into fp8 value + residual
        m2 = node_fts[bb] @ W2 + b2                    # [N, MID] f32
        m2q = m2.astype(F8NP)
        m2r = (m2 - m2q.astype(np.float32)).astype(F8NP)
        blocks = np.zeros((NCHUNK, 128, 128), dtype=F8NP)
        qi = np.arange(JD)
        blocks[:, 3 * qi + 0, :] = m2q.reshape(NCHUNK, JD, MID)
        blocks[:, 3 * qi + 1, :] = m2r.reshape(NCHUNK, JD, MID)
        blocks[:, 3 * qi + 2, :] = np.float32(MASK_NEG).astype(F8NP)
        wem = np.zeros((128, WEMW), dtype=F8NP)
        wem[:, 0:128] = We8
        wem[:, 128:] = np.ascontiguousarray(
            blocks.transpose(1, 0, 2)
        ).reshape(128, NCHUNK * 128)
        m["wem"] = wem

        # block-diagonal adjacency rows: gate / gate / 1-gate for sender q at
        # partitions 3q..3q+2, columns q*IH..(q+1)*IH; zeros elsewhere
        g = adj01[bb][:, sl]                           # [N, IH]
        gr = g.reshape(NCHUNK, JD, IH).astype(F8NP)
        gn = (1.0 - g).reshape(NCHUNK, JD, IH).astype(F8NP)
        adjf = np.zeros((NCHUNK, 3 * JD, CW), dtype=F8NP)
        for q in range(JD):
            adjf[:, 3 * q + 0, q * IH:(q + 1) * IH] = gr[:, q]
            adjf[:, 3 * q + 1, q * IH:(q + 1) * IH] = gr[:, q]
            adjf[:, 3 * q + 2, q * IH:(q + 1) * IH] = gn[:, q]
        m["adjf"] = adjf.reshape(NCHUNK * 3 * JD, CW)

        # receiver-side constant cT = (node@W1 + graph@Wg + b1+be+bg)^T
        cT = (node_fts[bb, sl] @ W1 + graph_fts[bb] @ Wg + b1 + be + bg).T
        fin = np.zeros((128, FINW), dtype=np.float16)
        fin[:, 0:256] = node_fts[bb, sl].T
        fin[:, 256:384] = Wo1
        fin[:, 384:512] = Wo2
        fin[:, 512:768] = cT
        fin[0, 768:896] = bo1 + bo2
        m["finpack"] = fin
        in_maps.append(m)

    res = run_bass_kernel_spmd(nc, in_maps, list(range(NCORES)))

    out = np.empty((B, N, OUT), dtype=np.float32)
    for c in range(NCORES):
        bb, ihh = c // 2, c % 2
        out[bb, ihh * IH:(ihh + 1) * IH, :] = res.results[c]["out"]
    return out


# revision 28
# speedup vs baseline: 1.0291x; 1.0243x over previous
"""Trainium2 Bass kernel for nn_Basic_MPNN — v6 (block-diagonal mask, balanced drains).

One fp8 DoubleRow matmul pair-span per 2 senders computes masked messages:
k-tile 0 contracts We with the edge chunk; k-tile 1 contracts a per-chunk
shared lhsT block (m2q/m2r/-224 rows for 16 senders at partitions 3q..3q+2)
against a block-diagonal adjacency rhs (gate/gate/1-gate at the same rows).
All m2/mask setup is host-packed: no on-device memsets of the weight side,
no on-device m2 matmuls.  Drains: Act copies 48 of 64 PSUM groups to f16
leaves (DVE folds them), DVE direct-folds the other 16 (max(acc, psum)),
sized from the cost model's LP optimum (Pool cannot read PSUM on TRN2 and
tensor ops allow only one PSUM operand, so these are the only two exits).
"""

import os
import sys

for _p in (
    "/root/.axon_site",
    "/root/.axon_site/_ro/trn_rl_repo",
    "/root/.axon_site/_ro/pypackages",
    "/opt/trn_rl_repo",
    "/opt/pypackages",
):
    if os.path.isdir(_p) and _p not in sys.path:
        sys.path.append(_p)

import numpy as np  # noqa: E402

import concourse.bass as bass  # noqa: E402
import concourse.tile as tile  # noqa: E402
from concourse import bacc, mybir  # noqa: E402
from concourse.ap import AP as BassAP  # noqa: E402
from concourse.bass_utils import run_bass_kernel_spmd  # noqa: E402

F32 = mybir.dt.float32
F16 = mybir.dt.float16
F8 = mybir.dt.float8e4

B, N, D, MID, OUT = 4, 512, 128, 128, 128
NCORES = 8
IH = N // 2            # receivers per core
JD = 16                # senders per edge chunk
NCHUNK = N // JD       # 32
CW = JD * IH           # 4096 edge cols per chunk
WEMW = 128 + NCHUNK * 128   # We + one 128-col m2 block per chunk
FINW = 896             # noderT(256) wo1(128) wo2(128) cT(256) row0:bso(128)
MASK_NEG = -224.0
EBUFS = 3
GSEND = 8              # senders per PSUM drain group
NGRP = N // GSEND      # 64
GW = GSEND * IH        # 2048 psum cols per group (8 senders x 256 receivers)
# drain classes (Pool cannot read PSUM; DVE ops allow only ONE PSUM operand):
#   AP: Act copy [2048]->f16 leaf, Pool folds into its acc chain
#   AD: Act copy, DVE folds
#   D:  DVE folds PSUM directly into f16 acc (max(acc, ps), fold inlined)
# LP balance from the cost model: 25 AP / 14 AD / 25 D.
_QUOTA = (("AD", 48.0), ("D", 16.0))


def _class_seq():
    acc = {k: 0.0 for k, _ in _QUOTA}
    seq = []
    for _ in range(NGRP):
        for k, q in _QUOTA:
            acc[k] += q / NGRP
        pick = max(acc, key=lambda k: acc[k])
        acc[pick] -= 1.0
        seq.append(pick)
    return seq


CLASS_SEQ = _class_seq()
LAST_AD = max(i for i, k in enumerate(CLASS_SEQ) if k == "AD")


def _build_program():
    nc = bacc.Bacc(
        "TRN2", target_bir_lowering=False, debug=False, num_devices=NCORES
    )

    edge = nc.dram_tensor("edge", [D, N, IH], F8, kind="ExternalInput").ap()
    adjf_d = nc.dram_tensor(
        "adjf", [NCHUNK * 3 * JD, CW], F8, kind="ExternalInput"
    ).ap()
    wem_d = nc.dram_tensor("wem", [128, WEMW], F8, kind="ExternalInput").ap()
    fin_d = nc.dram_tensor("finpack", [128, FINW], F16, kind="ExternalInput").ap()
    zeros_d = nc.dram_tensor("zeros", [128, CW], F8, kind="ExternalInput").ap()
    out_d = nc.dram_tensor("out", [IH, OUT], F32, kind="ExternalOutput").ap()

    with (
        tile.TileContext(nc) as tc,
        tc.tile_pool(name="persist", bufs=1) as pp,
        tc.tile_pool(name="edge", bufs=EBUFS) as ep,
        tc.tile_pool(name="leafA", bufs=8) as lAp,
        tc.tile_pool(name="accAD", bufs=2) as aDp,
        tc.tile_pool(name="accD", bufs=2) as aDDp,
        tc.tile_pool(name="fin", bufs=8) as fp,
        tc.tile_pool(name="ps2", bufs=2, space="PSUM") as psp,
    ):
        wem_sb = pp.tile([128, WEMW], F8)
        fin_sb = pp.tile([128, FINW], F16)
        ones16 = pp.tile([1, 128], F16)
        nc.gpsimd.memset(ones16[:], 1.0)

        noderT16 = fin_sb[:, 0:256]
        wo1_16 = fin_sb[:, 256:384]
        wo2_16 = fin_sb[:, 384:512]
        cT16 = fin_sb[:, 512:768]
        bso16 = fin_sb[0:1, 768:896]

        wb = wem_sb[:]
        pitch = wb.ap[0][0]

        # two independent fold chains on DVE
        accAD = [None]   # Act leaves folded by DVE
        accD = [None]    # PSUM folded directly by DVE
        preAD = [None]

        for c in range(NCHUNK):
            et = ep.tile([128, 2 * CW], F8, tag="e")
            # adjacency region init: off-diagonal + rows>=48 must be 0 once
            # per buffer (diagonal blocks are rewritten by every chunk's DMA)
            if c == 0:
                nc.scalar.memzero(et[:, CW:CW + 2048])
                nc.gpsimd.memset(et[:, CW + 2048:2 * CW], 0.0)
            elif c == 1:
                nc.gpsimd.memset(et[:, CW:CW + 2048], 0.0)
                nc.scalar.memzero(et[:, CW + 2048:2 * CW])
            elif c == 2:
                nc.sync.dma_start(et[:, CW:2 * CW], zeros_d[:, :])
            nc.sync.dma_start(
                et[:, 0:CW],
                edge[:, c * JD:(c + 1) * JD, :].rearrange("d j i -> d (j i)"),
            )
            ett = et[:]
            nc.sync.dma_start(
                et[0:3 * JD, CW:2 * CW],
                adjf_d[c * 3 * JD:(c + 1) * 3 * JD, :],
            )
            if c == 0:
                # after chunk-0 stream DMAs so the first edge tile lands early
                nc.sync.dma_start(wem_sb[:], wem_d[:, :])
            elif c == 16:
                nc.sync.dma_start(fin_sb[:], fin_d[:, :])
            et2 = ett.rearrange("d (t x) -> d t x", t=2)
            lhsT = BassAP(
                wb.tensor, wb.offset,
                [[pitch, 128], [128 * (c + 1), 2], [1, 128]],
            )
            for h in range(2):
                ps = psp.tile([128, GW], F32, tag="ps")
                for m in range(4):
                    s = h * GW + m * 512
                    nc.tensor.matmul(
                        ps[:, m * 512:(m + 1) * 512],
                        lhsT=lhsT,
                        rhs=et2[:, :, s:s + 512],
                        perf_mode=mybir.MatmulPerfMode.DoubleRow,
                        start=True, stop=True,
                    )
                g = 2 * c + h
                cls = CLASS_SEQ[g]
                if cls == "D":
                    nd = aDDp.tile([128, GW], F16, tag="accD")
                    if accD[0] is None:
                        nc.vector.tensor_copy(nd[:], ps[:])
                    else:
                        nc.vector.tensor_max(nd[:], accD[0][:], ps[:])
                    accD[0] = nd
                else:
                    if accAD[0] is None:
                        leaf = aDp.tile([128, GW], F16, tag="acc")
                        nc.scalar.copy(leaf[:], ps[:])
                        accAD[0] = leaf
                    else:
                        leaf = lAp.tile([128, GW], F16, tag="lA")
                        nc.scalar.copy(leaf[:], ps[:])
                        na = aDp.tile([128, GW], F16, tag="acc")
                        nc.vector.tensor_max(na[:], accAD[0][:], leaf[:])
                        accAD[0] = na
                if g == LAST_AD and g < NGRP - 1:
                    # pre-halve the Act acc while the trailing groups drain
                    tAD = fp.tile([128, 1024], F16, tag="fAD")
                    nc.vector.tensor_max(
                        tAD[:], accAD[0][:, 0:1024], accAD[0][:, 1024:2048]
                    )
                    preAD[0] = tAD

        # ---------------- merge + finalize ----------------
        if preAD[0] is None:
            preAD[0] = fp.tile([128, 1024], F16, tag="fAD", name="preAD0")
            nc.vector.tensor_max(
                preAD[0][:], accAD[0][:, 0:1024], accAD[0][:, 1024:2048]
            )
        m1 = fp.tile([128, 1024], F16, tag="f")
        nc.vector.tensor_max(m1[:], preAD[0][:], accD[0][:, 0:1024])
        m2 = fp.tile([128, 1024], F16, tag="f")
        nc.vector.tensor_max(m2[:], m1[:], accD[0][:, 1024:2048])
        t3 = fp.tile([128, 512], F16, tag="f")
        nc.vector.tensor_max(t3[:], m2[:, 0:512], m2[:, 512:1024])
        w = fp.tile([128, 256], F16, tag="f")
        nc.vector.tensor_max(w[:], t3[:, 0:256], t3[:, 256:512])
        msgs = fp.tile([128, 256], F16, tag="f")
        nc.vector.tensor_add(msgs[:], w[:], cT16)

        for ib in range(2):
            psf = psp.tile([128, GW], F32, tag="ps")
            ps_h = psf[:, 0:OUT]
            nc.tensor.matmul(
                ps_h, lhsT=msgs[:, ib * 128:(ib + 1) * 128],
                rhs=wo2_16, start=True, stop=False,
            )
            nc.tensor.matmul(
                ps_h, lhsT=noderT16[:, ib * 128:(ib + 1) * 128],
                rhs=wo1_16, start=False, stop=False,
            )
            nc.tensor.matmul(
                ps_h, lhsT=ones16[:, 0:128], rhs=bso16,
                start=False, stop=True,
            )
            o_sb = fp.tile([128, OUT], F32, tag="o")
            nc.scalar.activation(
                o_sb[:], ps_h, mybir.ActivationFunctionType.Relu
            )
            nc.sync.dma_start(out_d[ib * 128:(ib + 1) * 128, :], o_sb[:])

    nc.finalize()
    return nc


_CACHED = {}


def _get_program():
    if "nc" not in _CACHED:
        _CACHED["nc"] = _build_program()
    return _CACHED["nc"]


def kernel(**inputs) -> np.ndarray:
    import ml_dtypes
    F8NP = ml_dtypes.float8_e4m3

    nc = _get_program()

    def f32(x):
        return np.ascontiguousarray(np.asarray(x, dtype=np.float32))

    node_fts = f32(inputs["node_fts"])
    graph_fts = f32(inputs["graph_fts"])
    adj01 = np.asarray(inputs["adj_mat"]).astype(np.float32)
    edge8 = np.asarray(inputs["edge_fts"], dtype=F8NP)
    edgeT = edge8.transpose(0, 3, 1, 2)  # [B, D, j, i] view

    W1, b1 = f32(inputs["W1"]), f32(inputs["b1"])
    W2, b2 = f32(inputs["W2"]), f32(inputs["b2"])
    We8 = np.asarray(inputs["We"], dtype=F8NP)
    be = f32(inputs["be"])
    Wg, bg = f32(inputs["Wg"]), f32(inputs["bg"])
    Wo1, bo1 = f32(inputs["Wo1"]), f32(inputs["bo1"])
    Wo2, bo2 = f32(inputs["Wo2"]), f32(inputs["bo2"])

    zeros = np.zeros((128, CW), dtype=F8NP)
    in_maps = []
    for c in range(NCORES):
        bb, ihh = c // 2, c % 2
        sl = slice(ihh * IH, (ihh + 1) * IH)
        m = {}
        m["edge"] = np.ascontiguousarray(edgeT[bb, :, :, sl])
        m["zeros"] = zeros

        # m2 = sender-side message part; split into fp8 value + residual
        m2 = node_fts[bb] @ W2 + b2                    # [N, MID] f32
        m2q = m2.astype(F8NP)
        m2r = (m2 - m2q.astype(np.float32)).astype(F8NP)
        blocks = np.zeros((NCHUNK, 128, 128), dtype=F8NP)
        qi = np.arange(JD)
        blocks[:, 3 * qi + 0, :] = m2q.reshape(NCHUNK, JD, MID)
        blocks[:, 3 * qi + 1, :] = m2r.reshape(NCHUNK, JD, MID)
        blocks[:, 3 * qi + 2, :] = np.float32(MASK_NEG).astype(F8NP)
        wem = np.zeros((128, WEMW), dtype=F8NP)
        wem[:, 0:128] = We8
        wem[:, 128:] = np.ascontiguousarray(
            blocks.transpose(1, 0, 2)
        ).reshape(128, NCHUNK * 128)
        m["wem"] = wem

        # block-diagonal adjacency rows: gate / gate / 1-gate for sender q at
        # partitions 3q..3q+2, columns q*IH..(q+1)*IH; zeros elsewhere
        g = adj01[bb][:, sl]                           # [N, IH]
        gr = g.reshape(NCHUNK, JD, IH).astype(F8NP)
        gn = (1.0 - g).reshape(NCHUNK, JD, IH).astype(F8NP)
        adjf = np.zeros((NCHUNK, 3 * JD, CW), dtype=F8NP)
        for q in range(JD):
            adjf[:, 3 * q + 0, q * IH:(q + 1) * IH] = gr[:, q]
            adjf[:, 3 * q + 1, q * IH:(q + 1) * IH] = gr[:, q]
            adjf[:, 3 * q + 2, q * IH:(q + 1) * IH] = gn[:, q]
        m["adjf"] = adjf.reshape(NCHUNK * 3 * JD, CW)

        # receiver-side constant cT = (node@W1 + graph@Wg + b1+be+bg)^T
        cT = (node_fts[bb, sl] @ W1 + graph_fts[bb] @ Wg + b1 + be + bg).T
        fin = np.zeros((128, FINW), dtype=np.float16)
        fin[:, 0:256] = node_fts[bb, sl].T
        fin[:, 256:384] = Wo1
        fin[:, 384:512] = Wo2
        fin[:, 512:768] = cT
        fin[0, 768:896] = bo1 + bo2
        m["finpack"] = fin
        in_maps.append(m)

    res = run_bass_kernel_spmd(nc, in_maps, list(range(NCORES)))

    out = np.empty((B, N, OUT), dtype=np.float32)
    for c in range(NCORES):
        bb, ihh = c // 2, c % 2
        out[bb, ihh * IH:(ihh + 1) * IH, :] = res.results[c]["out"]
    return out


# revision 29
# speedup vs baseline: 1.0295x; 1.0004x over previous
"""Trainium2 Bass kernel for nn_Basic_MPNN — v6 (block-diagonal mask, balanced drains).

One fp8 DoubleRow matmul pair-span per 2 senders computes masked messages:
k-tile 0 contracts We with the edge chunk; k-tile 1 contracts a per-chunk
shared lhsT block (m2q/m2r/-224 rows for 16 senders at partitions 3q..3q+2)
against a block-diagonal adjacency rhs (gate/gate/1-gate at the same rows).
All m2/mask setup is host-packed: no on-device memsets of the weight side,
no on-device m2 matmuls.  Drains: Act copies 48 of 64 PSUM groups to f16
leaves (DVE folds them), DVE direct-folds the other 16 (max(acc, psum)),
sized from the cost model's LP optimum (Pool cannot read PSUM on TRN2 and
tensor ops allow only one PSUM operand, so these are the only two exits).
"""

import os
import sys

for _p in (
    "/root/.axon_site",
    "/root/.axon_site/_ro/trn_rl_repo",
    "/root/.axon_site/_ro/pypackages",
    "/opt/trn_rl_repo",
    "/opt/pypackages",
):
    if os.path.isdir(_p) and _p not in sys.path:
        sys.path.append(_p)

import numpy as np  # noqa: E402

import concourse.bass as bass  # noqa: E402
import concourse.tile as tile  # noqa: E402
from concourse import bacc, mybir  # noqa: E402
from concourse.ap import AP as BassAP  # noqa: E402
from concourse.bass_utils import run_bass_kernel_spmd  # noqa: E402

F32 = mybir.dt.float32
F16 = mybir.dt.float16
F8 = mybir.dt.float8e4

B, N, D, MID, OUT = 4, 512, 128, 128, 128
NCORES = 8
IH = N // 2            # receivers per core
JD = 16                # senders per edge chunk
NCHUNK = N // JD       # 32
CW = JD * IH           # 4096 edge cols per chunk
WEMW = 128 + NCHUNK * 128   # We + one 128-col m2 block per chunk
FINW = 896             # noderT(256) wo1(128) wo2(128) cT(256) row0:bso(128)
MASK_NEG = -224.0
EBUFS = 3
GSEND = 8              # senders per PSUM drain group
NGRP = N // GSEND      # 64
GW = GSEND * IH        # 2048 psum cols per group (8 senders x 256 receivers)
# drain classes (Pool cannot read PSUM; DVE ops allow only ONE PSUM operand):
#   AP: Act copy [2048]->f16 leaf, Pool folds into its acc chain
#   AD: Act copy, DVE folds
#   D:  DVE folds PSUM directly into f16 acc (max(acc, ps), fold inlined)
# LP balance from the cost model: 25 AP / 14 AD / 25 D.
_QUOTA = (("AD", 48.0), ("D", 16.0))


def _class_seq():
    acc = {k: 0.0 for k, _ in _QUOTA}
    seq = []
    for _ in range(NGRP):
        for k, q in _QUOTA:
            acc[k] += q / NGRP
        pick = max(acc, key=lambda k: acc[k])
        acc[pick] -= 1.0
        seq.append(pick)
    return seq


CLASS_SEQ = _class_seq()
LAST_AD = max(i for i, k in enumerate(CLASS_SEQ) if k == "AD")


def _build_program():
    nc = bacc.Bacc(
        "TRN2", target_bir_lowering=False, debug=False, num_devices=NCORES
    )

    edge = nc.dram_tensor("edge", [D, N, IH], F8, kind="ExternalInput").ap()
    adjf_d = nc.dram_tensor(
        "adjf", [NCHUNK * 3 * JD, CW], F8, kind="ExternalInput"
    ).ap()
    wem_d = nc.dram_tensor("wem", [128, WEMW], F8, kind="ExternalInput").ap()
    fin_d = nc.dram_tensor("finpack", [128, FINW], F16, kind="ExternalInput").ap()
    zeros_d = nc.dram_tensor("zeros", [128, CW], F8, kind="ExternalInput").ap()
    out_d = nc.dram_tensor("out", [IH, OUT], F32, kind="ExternalOutput").ap()

    with (
        tile.TileContext(nc) as tc,
        tc.tile_pool(name="persist", bufs=1) as pp,
        tc.tile_pool(name="edge", bufs=EBUFS) as ep,
        tc.tile_pool(name="leafA", bufs=8) as lAp,
        tc.tile_pool(name="accAD", bufs=2) as aDp,
        tc.tile_pool(name="accD", bufs=2) as aDDp,
        tc.tile_pool(name="fin", bufs=8) as fp,
        tc.tile_pool(name="ps2", bufs=2, space="PSUM") as psp,
    ):
        wem_sb = pp.tile([128, WEMW], F8)
        fin_sb = pp.tile([128, FINW], F16)
        ones16 = pp.tile([1, 128], F16)
        nc.gpsimd.memset(ones16[:], 1.0)

        noderT16 = fin_sb[:, 0:256]
        wo1_16 = fin_sb[:, 256:384]
        wo2_16 = fin_sb[:, 384:512]
        cT16 = fin_sb[:, 512:768]
        bso16 = fin_sb[0:1, 768:896]

        wb = wem_sb[:]
        pitch = wb.ap[0][0]

        # two independent fold chains on DVE
        accAD = [None]   # Act leaves folded by DVE
        accD = [None]    # PSUM folded directly by DVE
        preAD = [None]

        for c in range(NCHUNK):
            et = ep.tile([128, 2 * CW], F8, tag="e")
            # adjacency region init: off-diagonal + rows>=48 must be 0 once
            # per buffer (diagonal blocks are rewritten by every chunk's DMA)
            if c == 0:
                nc.scalar.memzero(et[:, CW:CW + 2048])
                nc.gpsimd.memset(et[:, CW + 2048:2 * CW], 0.0)
            elif c == 1:
                nc.gpsimd.memset(et[:, CW:CW + 2048], 0.0)
                nc.scalar.memzero(et[:, CW + 2048:2 * CW])
            elif c == 2:
                nc.sync.dma_start(et[:, CW:2 * CW], zeros_d[:, :])
            nc.sync.dma_start(
                et[:, 0:CW],
                edge[:, c * JD:(c + 1) * JD, :].rearrange("d j i -> d (j i)"),
            )
            ett = et[:]
            nc.sync.dma_start(
                et[0:3 * JD, CW:2 * CW],
                adjf_d[c * 3 * JD:(c + 1) * 3 * JD, :],
            )
            if c == 0:
                # split wem: chunk 0 only needs We + the first m2 blocks
                nc.sync.dma_start(wem_sb[:, 0:640], wem_d[:, 0:640])
                nc.sync.dma_start(wem_sb[:, 640:WEMW], wem_d[:, 640:WEMW])
            elif c == 16:
                nc.sync.dma_start(fin_sb[:], fin_d[:, :])
            et2 = ett.rearrange("d (t x) -> d t x", t=2)
            lhsT = BassAP(
                wb.tensor, wb.offset,
                [[pitch, 128], [128 * (c + 1), 2], [1, 128]],
            )
            for h in range(2):
                ps = psp.tile([128, GW], F32, tag="ps")
                for m in range(4):
                    s = h * GW + m * 512
                    nc.tensor.matmul(
                        ps[:, m * 512:(m + 1) * 512],
                        lhsT=lhsT,
                        rhs=et2[:, :, s:s + 512],
                        perf_mode=mybir.MatmulPerfMode.DoubleRow,
                        start=True, stop=True,
                    )
                g = 2 * c + h
                cls = CLASS_SEQ[g]
                if cls == "D":
                    nd = aDDp.tile([128, GW], F16, tag="accD")
                    if accD[0] is None:
                        nc.vector.tensor_copy(nd[:], ps[:])
                    else:
                        nc.vector.tensor_max(nd[:], accD[0][:], ps[:])
                    accD[0] = nd
                else:
                    if accAD[0] is None:
                        leaf = aDp.tile([128, GW], F16, tag="acc")
                        nc.scalar.copy(leaf[:], ps[:])
                        accAD[0] = leaf
                    else:
                        leaf = lAp.tile([128, GW], F16, tag="lA")
                        nc.scalar.copy(leaf[:], ps[:])
                        na = aDp.tile([128, GW], F16, tag="acc")
                        nc.vector.tensor_max(na[:], accAD[0][:], leaf[:])
                        accAD[0] = na
                if g == LAST_AD and g < NGRP - 1:
                    # pre-halve the Act acc while the trailing groups drain
                    tAD = fp.tile([128, 1024], F16, tag="fAD")
                    nc.vector.tensor_max(
                        tAD[:], accAD[0][:, 0:1024], accAD[0][:, 1024:2048]
                    )
                    preAD[0] = tAD

        # ---------------- merge + finalize ----------------
        if preAD[0] is None:
            preAD[0] = fp.tile([128, 1024], F16, tag="fAD", name="preAD0")
            nc.vector.tensor_max(
                preAD[0][:], accAD[0][:, 0:1024], accAD[0][:, 1024:2048]
            )
        m1 = fp.tile([128, 1024], F16, tag="f")
        nc.vector.tensor_max(m1[:], preAD[0][:], accD[0][:, 0:1024])
        m2 = fp.tile([128, 1024], F16, tag="f")
        nc.vector.tensor_max(m2[:], m1[:], accD[0][:, 1024:2048])
        t3 = fp.tile([128, 512], F16, tag="f")
        nc.vector.tensor_max(t3[:], m2[:, 0:512], m2[:, 512:1024])
        w = fp.tile([128, 256], F16, tag="f")
        nc.vector.tensor_max(w[:], t3[:, 0:256], t3[:, 256:512])
        msgs = fp.tile([128, 256], F16, tag="f")
        nc.vector.tensor_add(msgs[:], w[:], cT16)

        for ib in range(2):
            psf = psp.tile([128, GW], F32, tag="ps")
            ps_h = psf[:, 0:OUT]
            nc.tensor.matmul(
                ps_h, lhsT=msgs[:, ib * 128:(ib + 1) * 128],
                rhs=wo2_16, start=True, stop=False,
            )
            nc.tensor.matmul(
                ps_h, lhsT=noderT16[:, ib * 128:(ib + 1) * 128],
                rhs=wo1_16, start=False, stop=False,
            )
            nc.tensor.matmul(
                ps_h, lhsT=ones16[:, 0:128], rhs=bso16,
                start=False, stop=True,
            )
            o_sb = fp.tile([128, OUT], F32, tag="o")
            nc.scalar.activation(
                o_sb[:], ps_h, mybir.ActivationFunctionType.Relu
            )
            nc.sync.dma_start(out_d[ib * 128:(ib + 1) * 128, :], o_sb[:])

    nc.finalize()
    return nc


_CACHED = {}


def _get_program():
    if "nc" not in _CACHED:
        _CACHED["nc"] = _build_program()
    return _CACHED["nc"]


def kernel(**inputs) -> np.ndarray:
    import ml_dtypes
    F8NP = ml_dtypes.float8_e4m3

    nc = _get_program()

    def f32(x):
        return np.ascontiguousarray(np.asarray(x, dtype=np.float32))

    node_fts = f32(inputs["node_fts"])
    graph_fts = f32(inputs["graph_fts"])
    adj01 = np.asarray(inputs["adj_mat"]).astype(np.float32)
    edge8 = np.asarray(inputs["edge_fts"], dtype=F8NP)
    edgeT = edge8.transpose(0, 3, 1, 2)  # [B, D, j, i] view

    W1, b1 = f32(inputs["W1"]), f32(inputs["b1"])
    W2, b2 = f32(inputs["W2"]), f32(inputs["b2"])
    We8 = np.asarray(inputs["We"], dtype=F8NP)
    be = f32(inputs["be"])
    Wg, bg = f32(inputs["Wg"]), f32(inputs["bg"])
    Wo1, bo1 = f32(inputs["Wo1"]), f32(inputs["bo1"])
    Wo2, bo2 = f32(inputs["Wo2"]), f32(inputs["bo2"])

    zeros = np.zeros((128, CW), dtype=F8NP)
    in_maps = []
    for c in range(NCORES):
        bb, ihh = c // 2, c % 2
        sl = slice(ihh * IH, (ihh + 1) * IH)
        m = {}
        m["edge"] = np.ascontiguousarray(edgeT[bb, :, :, sl])
        m["zeros"] = zeros

        # m2 = sender-side message part; split into fp8 value + residual
        m2 = node_fts[bb] @ W2 + b2                    # [N, MID] f32
        m2q = m2.astype(F8NP)
        m2r = (m2 - m2q.astype(np.float32)).astype(F8NP)
        blocks = np.zeros((NCHUNK, 128, 128), dtype=F8NP)
        qi = np.arange(JD)
        blocks[:, 3 * qi + 0, :] = m2q.reshape(NCHUNK, JD, MID)
        blocks[:, 3 * qi + 1, :] = m2r.reshape(NCHUNK, JD, MID)
        blocks[:, 3 * qi + 2, :] = np.float32(MASK_NEG).astype(F8NP)
        wem = np.zeros((128, WEMW), dtype=F8NP)
        wem[:, 0:128] = We8
        wem[:, 128:] = np.ascontiguousarray(
            blocks.transpose(1, 0, 2)
        ).reshape(128, NCHUNK * 128)
        m["wem"] = wem

        # block-diagonal adjacency rows: gate / gate / 1-gate for sender q at
        # partitions 3q..3q+2, columns q*IH..(q+1)*IH; zeros elsewhere
        g = adj01[bb][:, sl]                           # [N, IH]
        gr = g.reshape(NCHUNK, JD, IH).astype(F8NP)
        gn = (1.0 - g).reshape(NCHUNK, JD, IH).astype(F8NP)
        adjf = np.zeros((NCHUNK, 3 * JD, CW), dtype=F8NP)
        for q in range(JD):
            adjf[:, 3 * q + 0, q * IH:(q + 1) * IH] = gr[:, q]
            adjf[:, 3 * q + 1, q * IH:(q + 1) * IH] = gr[:, q]
            adjf[:, 3 * q + 2, q * IH:(q + 1) * IH] = gn[:, q]
        m["adjf"] = adjf.reshape(NCHUNK * 3 * JD, CW)

        # receiver-side constant cT = (node@W1 + graph@Wg + b1+be+bg)^T
        cT = (node_fts[bb, sl] @ W1 + graph_fts[bb] @ Wg + b1 + be + bg).T
        fin = np.zeros((128, FINW), dtype=np.float16)
        fin[:, 0:256] = node_fts[bb, sl].T
        fin[:, 256:384] = Wo1
        fin[:, 384:512] = Wo2
        fin[:, 512:768] = cT
        fin[0, 768:896] = bo1 + bo2
        m["finpack"] = fin
        in_maps.append(m)

    res = run_bass_kernel_spmd(nc, in_maps, list(range(NCORES)))

    out = np.empty((B, N, OUT), dtype=np.float32)
    for c in range(NCORES):
        bb, ihh = c // 2, c % 2
        out[bb, ihh * IH:(ihh + 1) * IH, :] = res.results[c]["out"]
    return out
